# revision 10
# baseline (speedup 1.0000x reference)
"""Self-contained Trainium2 Bass kernel for a 2-layer GAT + BatchNorm + graph pooling.

Contract: kernel(**inputs) takes the FULL (unsharded) inputs and returns the
FULL [G, 1024] float32 output. Internally: shard nodes (and their incident
edges) across 8 NeuronCores, run one SPMD Bass program (AllGather for the
inter-layer feature table, AllReduce for BatchNorm stats), then combine the
per-core partial poolings on the host.

v2 design (vs the v1 baseline, which was GpSimd-bound at 4.3ms of gather
descriptor generation):
  - gather table rows are bf16 [h(256) | al_src(4) | pad] with 768B stride
    (was fp32 1280B): halves gather DMA bytes, 1.67x smaller AllGathers.
  - the per-edge al_dst gather (half of all gather descriptors) is gone:
    al_dst per edge-slot is computed on-device as K small selector-transpose
    matmuls (S01T^T @ al_dst_block) against the core-local al_dst block.
  - selector matrices S01 (slot-major) and S01T (dst-major) are precomputed
    on the host from the edge structure and shipped as bf16 inputs: no
    on-device selector generation at all, and the softmax-weighted
    aggregation matmuls run in bf16 (fp32 PSUM accumulate).
  - the 50000 appended self-loop edges never enter the gather streams: the
    self contribution exp(lrelu(als_i+ald_i)) * h_i is added per dst block
    from the core-resident dense-phase output (h_own).
  - per-block gather chunk counts are the max over the 8 cores of the actual
    per-core edge counts (was: global max over all blocks), cutting slot
    padding from 18% to ~8%.
  - BatchNorm statistics accumulate inline in the edge-1 epilogue (from the
    transposed output tiles), removing the separate stats pass over g1T.

Remaining per-core hot path: ~108k gathered rows per layer at ~7.8ns/row of
GpSimd descriptor generation -> ~1.9ms expected GpSimd busy, the roofline of
this algorithm shape on TRN2.
"""

import numpy as np
import ml_dtypes

import concourse.bass as bass
import concourse.bacc as bacc
import concourse.tile as tile
from concourse import mybir
from concourse import bass_utils
from concourse.masks import make_identity

F32 = mybir.dt.float32
BF16 = mybir.dt.bfloat16
I16 = mybir.dt.int16
ALU = mybir.AluOpType
ACTF = mybir.ActivationFunctionType

# problem constants (hardcoded per the harness contract)
N, F_IN, C0, C1, H, E, G = 50000, 128, 64, 64, 4, 800000, 256
HC = H * C0            # 256
NEG_SLOPE = 0.2
BN_EPS = 1e-5
NCORES = 8
NPC = N // NCORES      # nodes per core (6250)
SPLIT = 32768          # dma_gather int16 index limit -> split gather table
RW = 384               # bf16 gather-table row width (768 B, mult of 256 B)
PART = 128
PHASES = 6             # build phases 1..PHASES (bisection aid)


# --------------------------------------------------------------------------
# host-side preprocessing
# --------------------------------------------------------------------------

def _pack16(stream_i16, ncols):
    """dma_gather index layout: position i -> [i%16, i//16], replicated to
    partition groups 16k+p for the 8 Q7 cores."""
    base = stream_i16.reshape(ncols, 16).T          # [16, ncols]
    return np.tile(base, (8, 1)).astype(np.int16)   # [128, ncols]


def preprocess(x, edge_index, batch,
               W1, att_src1, att_dst1, b1, gamma, beta,
               W2, att_src2, att_dst2, b2):
    x = np.asarray(x, np.float32)
    edge_index = np.asarray(edge_index)
    batch = np.asarray(batch).astype(np.int64)
    W1 = np.asarray(W1, np.float32); W2 = np.asarray(W2, np.float32)

    # natural edges only; the appended self-loops are handled analytically
    src = edge_index[0].astype(np.int64)
    dst = edge_index[1].astype(np.int64)

    NB = (NPC + PART - 1) // PART                      # dst blocks per core

    # per (core, block) edge lists, sorted by local dst
    lo_srcs = [[None] * NB for _ in range(NCORES)]
    hi_srcs = [[None] * NB for _ in range(NCORES)]
    lo_ds = [[None] * NB for _ in range(NCORES)]
    hi_ds = [[None] * NB for _ in range(NCORES)]
    nlo = np.zeros((NCORES, NB), int); nhi = np.zeros((NCORES, NB), int)
    for r in range(NCORES):
        m = (dst >= r * NPC) & (dst < (r + 1) * NPC)
        s_r = src[m]
        dloc = dst[m] - r * NPC
        order = np.argsort(dloc, kind="stable")
        s_r = s_r[order]; dloc = dloc[order]
        blk = dloc // PART
        for b in range(NB):
            bm = blk == b
            sb_ = s_r[bm]; db_ = (dloc[bm] % PART)
            lo_m = sb_ < SPLIT
            lo_srcs[r][b] = sb_[lo_m]; lo_ds[r][b] = db_[lo_m]
            hi_srcs[r][b] = sb_[~lo_m] - SPLIT; hi_ds[r][b] = db_[~lo_m]
            nlo[r, b] = int(lo_m.sum()); nhi[r, b] = int((~lo_m).sum())

    klo_s = tuple(int(v) for v in np.ceil(nlo / PART).astype(int).max(axis=0))
    khi_s = tuple(int(v) for v in np.ceil(nhi / PART).astype(int).max(axis=0))
    KLOM = max(klo_s); KHIM = max(khi_s)
    KTM = max(a + b for a, b in zip(klo_s, khi_s))

    idx_lo = np.zeros((NCORES, NB, PART, KLOM * 8), np.int16)
    idx_hi = np.zeros((NCORES, NB, PART, KHIM * 8), np.int16)
    s01 = np.zeros((NCORES, NB, PART, KTM * PART), ml_dtypes.bfloat16)
    s01t = np.zeros((NCORES, NB, PART, KTM * PART), ml_dtypes.bfloat16)
    for r in range(NCORES):
        for b in range(NB):
            klo, khi = klo_s[b], khi_s[b]
            ls = np.zeros(klo * PART, np.int16); ls[:nlo[r, b]] = lo_srcs[r][b]
            hs = np.zeros(max(khi, 1) * PART, np.int16); hs[:nhi[r, b]] = hi_srcs[r][b]
            idx_lo[r, b, :, 0:klo * 8] = _pack16(ls, klo * 8)
            if khi:
                idx_hi[r, b, :, 0:khi * 8] = _pack16(hs, khi * 8)
            # selector matrices over the packed slots: slot (p, k) = stream
            # position k*128+p; lo stream at k in [0, klo), hi at [klo, klo+khi)
            pos_lo = np.arange(nlo[r, b]); pos_hi = np.arange(nhi[r, b])
            p_lo = pos_lo % PART; k_lo = pos_lo // PART
            p_hi = pos_hi % PART; k_hi = klo + pos_hi // PART
            d_lo = lo_ds[r][b]; d_hi = hi_ds[r][b]
            one = ml_dtypes.bfloat16(1.0)
            s01[r, b, p_lo, k_lo * PART + d_lo] = one
            s01[r, b, p_hi, k_hi * PART + d_hi] = one
            s01t[r, b, d_lo, k_lo * PART + p_lo] = one
            s01t[r, b, d_hi, k_hi * PART + p_hi] = one

    # batch-derived pooling metadata
    counts = np.bincount(batch, minlength=G).astype(np.float64)
    maskrow = np.zeros((NCORES, 1, NPC), np.float32)
    cinvrow = np.zeros((NCORES, 1, NPC), np.float32)
    lastcol = [dict() for _ in range(NCORES)]  # graph -> last own column
    for r in range(NCORES):
        bseg = batch[r * NPC:(r + 1) * NPC]
        same = np.ones(NPC, np.float32)
        same[0] = 0.0
        same[1:] = (bseg[1:] == bseg[:-1]).astype(np.float32)
        maskrow[r, 0] = same
        cinvrow[r, 0] = (1.0 / np.maximum(counts[bseg], 1.0)).astype(np.float32)
        gids, last_idx = np.unique(bseg[::-1], return_index=True)
        for g_, li in zip(gids, last_idx):
            lastcol[r][int(g_)] = NPC - 1 - int(li)

    # weight preprocessing (pure functions of weight inputs)
    def bmat(W, a_s, a_d, fin):
        Wr = W.reshape(fin, H, C0)
        bs = np.einsum("khc,hc->kh", Wr, np.asarray(a_s, np.float32))
        bd = np.einsum("khc,hc->kh", Wr, np.asarray(a_d, np.float32))
        return np.concatenate([bs, bd], axis=1).astype(np.float32)  # [fin, 8]

    B1 = bmat(W1, att_src1, att_dst1, F_IN)
    B2 = bmat(W2, att_src2, att_dst2, HC)

    shared = dict(
        W1=W1, B1=B1, W2=W2, B2=B2,
        b1row=np.asarray(b1, np.float32).reshape(2, PART),
        b2row=np.asarray(b2, np.float32).reshape(2, PART),
        gcol=np.asarray(gamma, np.float32).reshape(2, PART).T.copy(),
        bcol=np.asarray(beta, np.float32).reshape(2, PART).T.copy(),
    )
    in_maps = []
    for r in range(NCORES):
        in_maps.append(dict(
            shared,
            xT=np.ascontiguousarray(x[r * NPC:(r + 1) * NPC].T),
            idx_lo=idx_lo[r],
            idx_hi=idx_hi[r],
            s01=s01[r],
            s01t=s01t[r],
            maskrow=maskrow[r],
            cinvrow=cinvrow[r],
        ))
    meta = dict(NB=NB, klo_s=klo_s, khi_s=khi_s, KLOM=KLOM, KHIM=KHIM,
                KTM=KTM, lastcol=lastcol, counts=counts)
    return in_maps, meta


def _cache_key(meta):
    return (meta["NB"], meta["klo_s"], meta["khi_s"])


# --------------------------------------------------------------------------
# device program
# --------------------------------------------------------------------------

def build_program(meta):
    NB, klo_s, khi_s = meta["NB"], meta["klo_s"], meta["khi_s"]
    KLOM, KHIM, KTM = meta["KLOM"], meta["KHIM"], meta["KTM"]
    nc = bacc.Bacc("TRN2", target_bir_lowering=False, debug=False,
                   num_devices=NCORES)

    def ein(name, shape, dt=F32):
        return nc.dram_tensor(name, list(shape), dt, kind="ExternalInput").ap()

    xT_d = ein("xT", [F_IN, NPC])
    W1_d = ein("W1", [F_IN, HC]);  B1_d = ein("B1", [F_IN, 8])
    W2_d = ein("W2", [HC, HC]);    B2_d = ein("B2", [HC, 8])
    b1r_d = ein("b1row", [2, PART]); b2r_d = ein("b2row", [2, PART])
    gcol_d = ein("gcol", [PART, 2]); bcol_d = ein("bcol", [PART, 2])
    ilo_d = ein("idx_lo", [NB, PART, KLOM * 8], I16)
    ihi_d = ein("idx_hi", [NB, PART, KHIM * 8], I16)
    s01_d = ein("s01", [NB, PART, KTM * PART], BF16)
    s01t_d = ein("s01t", [NB, PART, KTM * PART], BF16)
    mask_d = ein("maskrow", [1, NPC])
    cinv_d = ein("cinvrow", [1, NPC])

    omax_d = nc.dram_tensor("out_max", [4 * PART, NPC], F32, kind="ExternalOutput").ap()
    omean_d = nc.dram_tensor("out_mean", [4 * PART, NPC], F32, kind="ExternalOutput").ap()

    # internal DRAM
    ag1_in = nc.dram_tensor("ag1_in", [NPC, RW], BF16).ap()
    T1 = nc.dram_tensor("T1", [N, RW], BF16, addr_space="Shared").ap()
    ag2_in = nc.dram_tensor("ag2_in", [NPC, RW], BF16).ap()
    T2 = nc.dram_tensor("T2", [N, RW], BF16, addr_space="Shared").ap()
    g1T = nc.dram_tensor("g1T", [HC, NPC], F32).ap()
    x2T = nc.dram_tensor("x2T", [HC, NPC], F32).ap()
    ar_in = nc.dram_tensor("ar_in", [PART, 4], F32).ap()
    ar_out = nc.dram_tensor("ar_out", [PART, 4], F32, addr_space="Shared").ap()

    rgroups = [list(range(NCORES))]

    class _PhaseStopE(Exception):
        pass

    with tile.TileContext(nc) as tc:
      try:
        # ---------- shared constant tiles ----------
        with tc.tile_pool(name="const", bufs=1) as cpool:
            # identity for PE transposes, with a ones column at 128 so the
            # same matmul also emits per-channel block sums (BN statistics).
            ident = cpool.tile([PART, PART + 4], F32)
            make_identity(nc, ident[:, 0:PART])
            nc.vector.memset(ident[:, PART:PART + 4], 0.0)
            nc.vector.memset(ident[:, PART:PART + 1], 1.0)

            def bias_bcast(row_d, pool, psum_pool, tag):
                bv = pool.tile([PART, 2], F32, tag=f"biasv{tag}")
                nc.sync.dma_start(out=bv[:], in_=row_d[:, :].rearrange("c p -> p c"))
                bb = pool.tile([PART, HC], F32, tag=f"biasb{tag}")
                for c in range(2):
                    tp = psum_pool.tile([PART, PART], F32, tag=f"biastp{tag}{c}")
                    nc.tensor.transpose(out=tp[:], in_=bv[:, c:c + 1].to_broadcast([PART, PART]),
                                        identity=ident[:, 0:PART])
                    nc.vector.tensor_copy(out=bb[:, c * PART:(c + 1) * PART], in_=tp[:])
                return bb

            with tc.tile_pool(name="biasps", bufs=1, space="PSUM") as bps:
                b1b = bias_bcast(b1r_d, cpool, bps, "1")
                b2b = bias_bcast(b2r_d, cpool, bps, "2")

            # ---------- edge phase (shared for both layers) ----------
            def edge_phase(Tbl, h_own, bbias, outT, relu, statsS, statsQ):
                with tc.tile_pool(name="eidx", bufs=2) as ip, \
                     tc.tile_pool(name="eg", bufs=2) as gp, \
                     tc.tile_pool(name="esel", bufs=2) as sp_, \
                     tc.tile_pool(name="ew", bufs=2) as wp2, \
                     tc.tile_pool(name="eps", bufs=2, space="PSUM") as ep, \
                     tc.tile_pool(name="ealdps", bufs=2, space="PSUM") as ap_, \
                     tc.tile_pool(name="esps", bufs=2, space="PSUM") as sps, \
                     tc.tile_pool(name="etps", bufs=2, space="PSUM") as tps:
                    for b in range(NB):
                        mb = min(PART, NPC - b * PART)
                        klo, khi = klo_s[b], khi_s[b]
                        KT = klo + khi
                        il = ip.tile([PART, KLOM * 8], I16, tag="il")
                        nc.sync.dma_start(out=il[:, 0:klo * 8], in_=ilo_d[b, :, 0:klo * 8])
                        ih = ip.tile([PART, KHIM * 8], I16, tag="ih")
                        nc.sync.dma_start(out=ih[:, 0:khi * 8], in_=ihi_d[b, :, 0:khi * 8])
                        s01 = sp_.tile([PART, KTM * PART], BF16, tag="s01")
                        nc.sync.dma_start(out=s01[:, 0:KT * PART], in_=s01_d[b, :, 0:KT * PART])
                        s01t = sp_.tile([PART, KTM * PART], BF16, tag="s01t")
                        nc.sync.dma_start(out=s01t[:, 0:KT * PART], in_=s01t_d[b, :, 0:KT * PART])

                        # dma_gather is limited to 1024 indices per
                        # instruction (HW hang beyond that) -> chunk by 8
                        # 128-row blocks.
                        glo = gp.tile([PART, KTM, RW], BF16, tag="glo")
                        for c0 in range(0, klo, 8):
                            cnt = min(8, klo - c0)
                            nc.gpsimd.dma_gather(
                                out_ap=glo[:, c0:c0 + cnt, :],
                                in_ap=Tbl[:, :], idxs_ap=il[:, c0 * 8:(c0 + cnt) * 8],
                                num_idxs=cnt * PART, num_idxs_reg=cnt * PART,
                                elem_size=RW)
                        for c0 in range(0, khi, 8):
                            cnt = min(8, khi - c0)
                            nc.gpsimd.dma_gather(
                                out_ap=glo[:, klo + c0:klo + c0 + cnt, :],
                                in_ap=Tbl[SPLIT:N, :], idxs_ap=ih[:, c0 * 8:(c0 + cnt) * 8],
                                num_idxs=cnt * PART, num_idxs_reg=cnt * PART,
                                elem_size=RW)

                        # al_dst per edge slot: K small selector-transpose
                        # matmuls against the core-local al_dst block.
                        aldps = ap_.tile([PART, KTM * 4], F32, tag="aldps")
                        for k in range(KT):
                            nc.tensor.matmul(aldps[:, k * 4:(k + 1) * 4],
                                             lhsT=s01t[:, k * PART:(k + 1) * PART],
                                             rhs=h_own[:, b, 260:264],
                                             start=True, stop=True)

                        # z = al_src[src] + al_dst[dst];
                        # exp(leaky_relu(z)) = max(exp(z), exp(0.2*z))
                        z = wp2.tile([PART, KTM, 4], F32, tag="z")
                        nc.vector.tensor_tensor(
                            out=z[:, 0:KT, :],
                            in0=glo[:, 0:KT, 256:260],
                            in1=aldps[:, 0:KT * 4].rearrange("p (k h) -> p k h", k=KT),
                            op=ALU.add)
                        e1 = wp2.tile([PART, KTM, 4], F32, tag="e1")
                        nc.scalar.activation(out=e1[:, 0:KT, :], in_=z[:, 0:KT, :],
                                             func=ACTF.Exp)
                        e2 = wp2.tile([PART, KTM, 4], F32, tag="e2")
                        nc.scalar.activation(out=e2[:, 0:KT, :], in_=z[:, 0:KT, :],
                                             func=ACTF.Exp, scale=NEG_SLOPE)
                        # hp layout: [exp(4) | h*exp(256)]
                        hp = wp2.tile([PART, KTM, 260], BF16, tag="hp")
                        nc.vector.tensor_tensor(
                            out=hp[:, 0:KT, 0:4],
                            in0=e1[:, 0:KT, :], in1=e2[:, 0:KT, :],
                            op=ALU.max)
                        nc.vector.tensor_tensor(
                            out=hp[:, 0:KT, 4:260].rearrange("p k (h c) -> p k h c", h=H),
                            in0=glo[:, 0:KT, 0:HC].rearrange("p k (h c) -> p k h c", h=H),
                            in1=hp[:, 0:KT, 0:4].unsqueeze(-1).to_broadcast([PART, KT, H, C0]),
                            op=ALU.mult)

                        # aggregate: acc[dst, exp|(h c)] via bf16 selector matmuls
                        acc = ep.tile([PART, 260], F32, tag="acc")
                        for k in range(KT):
                            nc.tensor.matmul(acc[:], lhsT=s01[:, k * PART:(k + 1) * PART],
                                             rhs=hp[:, k, 0:260],
                                             start=(k == 0), stop=(k == KT - 1))

                        # self-loop contribution (handled outside the gather
                        # streams): w = exp(lrelu(als_i + ald_i)), msg w*h_i
                        zs = wp2.tile([PART, 4], F32, tag="zs")
                        nc.vector.tensor_tensor(out=zs[:], in0=h_own[:, b, 256:260],
                                                in1=h_own[:, b, 260:264], op=ALU.add)
                        e1s = wp2.tile([PART, 4], F32, tag="e1s")
                        nc.scalar.activation(out=e1s[:], in_=zs[:], func=ACTF.Exp)
                        e2s = wp2.tile([PART, 4], F32, tag="e2s")
                        nc.scalar.activation(out=e2s[:], in_=zs[:], func=ACTF.Exp,
                                             scale=NEG_SLOPE)
                        hws = wp2.tile([PART, 260], F32, tag="hws")
                        nc.vector.tensor_tensor(out=hws[:, 0:4], in0=e1s[:], in1=e2s[:],
                                                op=ALU.max)
                        nc.vector.tensor_tensor(
                            out=hws[:, 4:260].rearrange("p (h c) -> p h c", h=H),
                            in0=h_own[:, b, 0:HC].rearrange("p (h c) -> p h c", h=H),
                            in1=hws[:, 0:4].unsqueeze(-1).to_broadcast([PART, H, C0]),
                            op=ALU.mult)
                        num = wp2.tile([PART, 260], F32, tag="num")
                        nc.vector.tensor_tensor(out=num[:], in0=acc[:], in1=hws[:],
                                                op=ALU.add)
                        # denominator >= self-loop weight > 0, so the
                        # reference's +1e-16 guard is a no-op here
                        rec = wp2.tile([PART, 4], F32, tag="rec")
                        nc.vector.reciprocal(out=rec[:], in_=num[:, 0:4])
                        ob = wp2.tile([PART, HC], F32, tag="ob")
                        nc.vector.tensor_tensor(
                            out=ob[:].rearrange("p (h c) -> p h c", h=H),
                            in0=num[:, 4:260].rearrange("p (h c) -> p h c", h=H),
                            in1=rec[:].unsqueeze(-1).to_broadcast([PART, H, C0]),
                            op=ALU.mult)
                        nc.vector.tensor_tensor(out=ob[:], in0=ob[:], in1=bbias[:], op=ALU.add)
                        if relu:
                            nc.vector.tensor_scalar_max(out=ob[:], in0=ob[:], scalar1=0.0)
                        if statsS is not None:
                            obsq = wp2.tile([PART, HC], F32, tag="obsq")
                            nc.scalar.activation(out=obsq[0:mb, :], in_=ob[0:mb, :],
                                                 func=ACTF.Square)
                            sqs = sps.tile([PART, 4], F32, tag="sqs")
                        for c in range(2):
                            # transpose via matmul; the ones column at 128
                            # emits the per-channel block sum for BN stats
                            ncols = 129 if statsS is not None else PART
                            tp = tps.tile([PART, PART + 4], F32, tag="ttp")
                            nc.tensor.matmul(tp[:, 0:ncols],
                                             lhsT=ob[0:mb, c * PART:(c + 1) * PART],
                                             rhs=ident[0:mb, 0:ncols],
                                             start=True, stop=True)
                            tsb = wp2.tile([PART, PART], F32, tag="tsb")
                            nc.vector.tensor_copy(out=tsb[:, 0:mb], in_=tp[:, 0:mb])
                            nc.sync.dma_start(
                                out=outT[c * PART:(c + 1) * PART, b * PART:b * PART + mb],
                                in_=tsb[:, 0:mb])
                            if statsS is not None:
                                nc.vector.tensor_tensor(out=statsS[:, c:c + 1],
                                                        in0=statsS[:, c:c + 1],
                                                        in1=tp[:, PART:PART + 1], op=ALU.add)
                                nc.tensor.matmul(sqs[:, c:c + 1],
                                                 lhsT=obsq[0:mb, c * PART:(c + 1) * PART],
                                                 rhs=ident[0:mb, PART:PART + 1],
                                                 start=True, stop=True)
                                nc.vector.tensor_tensor(out=statsQ[:, c:c + 1],
                                                        in0=statsQ[:, c:c + 1],
                                                        in1=sqs[:, c:c + 1], op=ALU.add)

            # ---------- layer 1 ----------
            with tc.tile_pool(name="L1res", bufs=1) as h1pool:
                h_own1 = h1pool.tile([PART, NB, 264], BF16)
                nc.vector.memset(h_own1[:], 0.0)
                statsS = h1pool.tile([PART, 2], F32, tag="statsS")
                nc.vector.memset(statsS[:], 0.0)
                statsQ = h1pool.tile([PART, 2], F32, tag="statsQ")
                nc.vector.memset(statsQ[:], 0.0)

                with tc.tile_pool(name="d1w", bufs=1) as wp, \
                     tc.tile_pool(name="d1ps", bufs=2, space="PSUM") as pp:
                    xT_sb = wp.tile([F_IN, NPC], F32)
                    nc.sync.dma_start(out=xT_sb[:], in_=xT_d[:, :])
                    W1_sb = wp.tile([F_IN, HC], F32)
                    nc.sync.dma_start(out=W1_sb[:], in_=W1_d[:, :])
                    B1_sb = wp.tile([F_IN, 8], F32)
                    nc.sync.dma_start(out=B1_sb[:], in_=B1_d[:, :])
                    for b in range(NB):
                        mb = min(PART, NPC - b * PART)
                        ps = pp.tile([PART, 264], F32, tag="dps")
                        nc.tensor.matmul(ps[0:mb, 0:HC], lhsT=xT_sb[:, b * PART:b * PART + mb],
                                         rhs=W1_sb[:], start=True, stop=True)
                        nc.tensor.matmul(ps[0:mb, HC:HC + 8], lhsT=xT_sb[:, b * PART:b * PART + mb],
                                         rhs=B1_sb[:], start=True, stop=True)
                        nc.vector.tensor_copy(out=h_own1[0:mb, b, :], in_=ps[0:mb, :])
                        nc.sync.dma_start(out=ag1_in[b * PART:b * PART + mb, 0:264],
                                          in_=h_own1[0:mb, b, 0:264])

                nc.gpsimd.collective_compute(
                    "AllGather", ALU.bypass, replica_groups=rgroups,
                    ins=[ag1_in[:, :]], outs=[T1[:, :]])

                if PHASES < 2:
                    raise _PhaseStopE
                edge_phase(T1, h_own1, b1b, g1T, relu=False,
                           statsS=statsS, statsQ=statsQ)

                if PHASES < 3:
                    raise _PhaseStopE
                with tc.tile_pool(name="stw", bufs=1) as sw:
                    stats = sw.tile([PART, 4], F32)
                    nc.vector.tensor_copy(out=stats[:, 0:2], in_=statsS[:])
                    nc.vector.tensor_copy(out=stats[:, 2:4], in_=statsQ[:])
                    nc.sync.dma_start(out=ar_in[:, :], in_=stats[:])

            nc.gpsimd.collective_compute(
                "AllReduce", ALU.add, replica_groups=rgroups,
                ins=[ar_in[:, :]], outs=[ar_out[:, :]])

            with tc.tile_pool(name="bnw", bufs=1) as bw:
                ar_sb = bw.tile([PART, 4], F32)
                nc.sync.dma_start(out=ar_sb[:], in_=ar_out[:, :])
                mean = bw.tile([PART, 2], F32)
                nc.vector.tensor_scalar_mul(out=mean[:], in0=ar_sb[:, 0:2], scalar1=1.0 / N)
                msq = bw.tile([PART, 2], F32)
                nc.vector.tensor_scalar_mul(out=msq[:], in0=ar_sb[:, 2:4], scalar1=1.0 / N)
                var = bw.tile([PART, 2], F32)
                nc.vector.tensor_tensor(out=var[:], in0=mean[:], in1=mean[:], op=ALU.mult)
                nc.vector.tensor_tensor(out=var[:], in0=msq[:], in1=var[:], op=ALU.subtract)
                nc.vector.tensor_scalar_add(out=var[:], in0=var[:], scalar1=BN_EPS)
                sd = bw.tile([PART, 2], F32)
                nc.scalar.activation(out=sd[:], in_=var[:], func=ACTF.Sqrt)
                rinv = bw.tile([PART, 2], F32)
                nc.vector.reciprocal(out=rinv[:], in_=sd[:])
                gc = bw.tile([PART, 2], F32)
                nc.sync.dma_start(out=gc[:], in_=gcol_d[:, :])
                bc = bw.tile([PART, 2], F32)
                nc.sync.dma_start(out=bc[:], in_=bcol_d[:, :])
                scale_c = bw.tile([PART, 2], F32)
                nc.vector.tensor_tensor(out=scale_c[:], in0=gc[:], in1=rinv[:], op=ALU.mult)
                shift_c = bw.tile([PART, 2], F32)
                nc.vector.tensor_tensor(out=shift_c[:], in0=mean[:], in1=scale_c[:], op=ALU.mult)
                nc.vector.tensor_tensor(out=shift_c[:], in0=bc[:], in1=shift_c[:], op=ALU.subtract)

                # ---------- layer 2 ----------
                if PHASES < 4:
                    raise _PhaseStopE
                with tc.tile_pool(name="L2res", bufs=1) as h2pool:
                    h_own2 = h2pool.tile([PART, NB, 264], BF16)
                    nc.vector.memset(h_own2[:], 0.0)
                    with tc.tile_pool(name="d2", bufs=2) as dp2, \
                         tc.tile_pool(name="d2w", bufs=1) as wp3, \
                         tc.tile_pool(name="d2ps", bufs=2, space="PSUM") as pp2:
                        W2_sb = [wp3.tile([PART, HC], F32, tag=f"w2_{kt}", name=f"w2_{kt}")
                                 for kt in range(2)]
                        B2_sb = [wp3.tile([PART, 8], F32, tag=f"b2_{kt}", name=f"b2_{kt}")
                                 for kt in range(2)]
                        for kt in range(2):
                            nc.sync.dma_start(out=W2_sb[kt][:],
                                              in_=W2_d[kt * PART:(kt + 1) * PART, :])
                            nc.sync.dma_start(out=B2_sb[kt][:],
                                              in_=B2_d[kt * PART:(kt + 1) * PART, :])
                        for b in range(NB):
                            mb = min(PART, NPC - b * PART)
                            ps = pp2.tile([PART, 264], F32, tag="d2psacc")
                            x1s_l = []
                            for kt in range(2):
                                gsl = dp2.tile([PART, PART], F32, tag="gsl")
                                nc.sync.dma_start(
                                    out=gsl[:, 0:mb],
                                    in_=g1T[kt * PART:(kt + 1) * PART, b * PART:b * PART + mb])
                                x1s = dp2.tile([PART, PART], F32, tag="x1s")
                                nc.scalar.activation(out=x1s[:, 0:mb], in_=gsl[:, 0:mb],
                                                     func=ACTF.Relu,
                                                     bias=shift_c[:, kt:kt + 1],
                                                     scale=scale_c[:, kt:kt + 1])
                                x1s_l.append(x1s)
                            # NOTE: start=True clears has_written for the whole
                            # PSUM bank, so each region's accumulation group must
                            # finish before the next region starts.
                            for kt in range(2):
                                nc.tensor.matmul(ps[0:mb, 0:HC], lhsT=x1s_l[kt][:, 0:mb],
                                                 rhs=W2_sb[kt][:],
                                                 start=(kt == 0), stop=(kt == 1))
                            for kt in range(2):
                                nc.tensor.matmul(ps[0:mb, HC:HC + 8], lhsT=x1s_l[kt][:, 0:mb],
                                                 rhs=B2_sb[kt][:],
                                                 start=(kt == 0), stop=(kt == 1))
                            nc.vector.tensor_copy(out=h_own2[0:mb, b, :], in_=ps[0:mb, :])
                            nc.sync.dma_start(out=ag2_in[b * PART:b * PART + mb, 0:264],
                                              in_=h_own2[0:mb, b, 0:264])

                    def pool_cts(pl, cts):
                        mk = pl.tile([PART, NPC], F32, tag="mk")
                        nc.sync.dma_start(out=mk[:], in_=mask_d[0:1, :].to_broadcast([PART, NPC]))
                        cv = pl.tile([PART, NPC], F32, tag="cv")
                        nc.sync.dma_start(out=cv[:], in_=cinv_d[0:1, :].to_broadcast([PART, NPC]))
                        for ct in cts:
                            xt = pl.tile([PART, NPC], F32, tag="xt")
                            if ct < 2:
                                gld = pl.tile([PART, NPC], F32, tag="gld")
                                nc.sync.dma_start(out=gld[:], in_=g1T[ct * PART:(ct + 1) * PART, :])
                                nc.scalar.activation(out=xt[:], in_=gld[:], func=ACTF.Relu,
                                                     bias=shift_c[:, ct:ct + 1],
                                                     scale=scale_c[:, ct:ct + 1])
                            else:
                                nc.sync.dma_start(out=xt[:],
                                                  in_=x2T[(ct - 2) * PART:(ct - 1) * PART, :])
                            sm = pl.tile([PART, NPC], F32, tag="sm")
                            nc.vector.tensor_tensor_scan(out=sm[:], data0=mk[:], data1=xt[:],
                                                         initial=0.0, op0=ALU.mult, op1=ALU.max)
                            nc.sync.dma_start(out=omax_d[ct * PART:(ct + 1) * PART, :], in_=sm[:])
                            ss = pl.tile([PART, NPC], F32, tag="ss")
                            nc.vector.tensor_tensor_scan(out=ss[:], data0=mk[:], data1=xt[:],
                                                         initial=0.0, op0=ALU.mult, op1=ALU.add)
                            nc.vector.tensor_tensor(out=ss[:], in0=ss[:], in1=cv[:], op=ALU.mult)
                            nc.sync.dma_start(out=omean_d[ct * PART:(ct + 1) * PART, :], in_=ss[:])

                    # pooling of the layer-1 channels overlaps the T2 AllGather
                    if PHASES >= 6:
                        with tc.tile_pool(name="pl1", bufs=1) as pl:
                            pool_cts(pl, (0, 1))

                    nc.gpsimd.collective_compute(
                        "AllGather", ALU.bypass, replica_groups=rgroups,
                        ins=[ag2_in[:, :]], outs=[T2[:, :]])

                    if PHASES < 5:
                        raise _PhaseStopE
                    edge_phase(T2, h_own2, b2b, x2T, relu=True,
                               statsS=None, statsQ=None)

                    # ---------- pooling (layer-2 channels) ----------
                    if PHASES < 6:
                        raise _PhaseStopE
                    with tc.tile_pool(name="pl2", bufs=1) as pl:
                        pool_cts(pl, (2, 3))

      except _PhaseStopE:
        pass

    nc.compile()
    return nc


# --------------------------------------------------------------------------
# host-side combine
# --------------------------------------------------------------------------

def postprocess(results, meta):
    lastcol = meta["lastcol"]
    mean = np.zeros((G, 2 * HC), np.float32)
    mx = np.zeros((G, 2 * HC), np.float32)
    for r in range(NCORES):
        om = results[r]["out_mean"]   # [512, NPC]
        ox = results[r]["out_max"]
        for g_, col in lastcol[r].items():
            mean[g_] += om[:, col]
            mx[g_] = np.maximum(mx[g_], ox[:, col])
    # empty graphs stay 0 (matches reference semantics)
    return np.concatenate([mean, mx], axis=1).astype(np.float32)


_CACHE = {}


def kernel(**inputs):
    in_maps, meta = preprocess(**inputs)
    key = _cache_key(meta)
    if key not in _CACHE:
        _CACHE[key] = build_program(meta)
    nc = _CACHE[key]
    res = bass_utils.run_bass_kernel_spmd(nc, in_maps, core_ids=list(range(NCORES)))
    return postprocess(res.results, meta)
